# revision 1
# baseline (speedup 1.0000x reference)
"""GCN (2x GCNConv + ReLU, then Linear+PReLU+Linear) on 8 Trainium2 NeuronCores.

Sharding: destination-node range partitioning (12500 nodes/core). Each core
builds the full layer-1 table t1 = (dinv*x)@W1 redundantly (x is replicated),
gathers source rows for its own edges (sorted by dst, chunked so each
1024-edge chunk spans <=128 dst nodes), segment-sums via one-hot matmuls in
PSUM, applies dinv scaling + bias + relu.  The layer-2 table shard
t2 = (dinv*z1)@W2 is produced fused into the conv1 epilogue and AllGathered.
Conv2 runs the same way producing the z shard, then the projection head.
Tables are bf16 (halves gather traffic); accumulation is f32 in PSUM.
"""

import sys

sys.path.insert(0, "/opt/trn_rl_repo")

import numpy as np
import ml_dtypes

import concourse.bass as bass
import concourse.bacc as bacc
import concourse.tile as tile
from concourse import mybir
from concourse.bass_utils import run_bass_kernel_spmd

N = 100000
E = 1000000
D = 128
NCORES = 8
SHARD = N // NCORES          # 12500
SLOTS = 12544                # 98 * 128 per-core slot space (padded)
NBLK = SLOTS // 128          # 98
TROWS = NCORES * SLOTS       # 100352 table rows
TRASH = 12600                # scatter target for unused psum rows
LBUF = 12672                 # per-core shard buffer rows
CHUNK = 1024                 # edges per chunk (8 tiles of 128)
G = 16                       # chunks per constant-load group

BF16 = mybir.dt.bfloat16
F32 = mybir.dt.float32
I32 = mybir.dt.int32
BF = ml_dtypes.bfloat16


def _slot(v):
    return (v // SHARD) * SLOTS + (v % SHARD)


def _preprocess(edge_index):
    """Host-side graph structure prep. Returns dinv + per-core constant arrays."""
    src = np.asarray(edge_index[0], dtype=np.int64)
    dst = np.asarray(edge_index[1], dtype=np.int64)
    deg = np.bincount(dst, minlength=N).astype(np.float64) + 1.0  # + self loop
    dinv = (1.0 / np.sqrt(deg)).astype(np.float32)
    rdinv = np.sqrt(deg).astype(np.float32)

    cores = []
    for c in range(NCORES):
        v0, v1 = c * SHARD, (c + 1) * SHARD
        m = (dst >= v0) & (dst < v1)
        s_c, d_c = src[m], dst[m]
        loops = np.arange(v0, v1, dtype=np.int64)
        s_all = np.concatenate([s_c, loops])
        d_all = np.concatenate([d_c, loops])
        order = np.argsort(d_all, kind="stable")
        s_sorted = s_all[order]
        cnt = np.bincount(d_all - v0, minlength=SHARD)  # per-node edges (>=1)
        rp = np.zeros(SHARD + 1, dtype=np.int64)
        np.cumsum(cnt, out=rp[1:])
        chunks = []
        lo = 0
        while lo < SHARD:
            hi, ne = lo, 0
            while hi < SHARD and hi - lo < 128 and ne + cnt[hi] <= CHUNK:
                ne += cnt[hi]
                hi += 1
            assert hi > lo, f"node {lo} degree {cnt[lo]} exceeds chunk size"
            chunks.append((lo, hi))
            lo = hi
        cores.append((chunks, s_sorted, rp, cnt))

    nch = max(len(c[0]) for c in cores)
    nch = ((nch + G - 1) // G) * G
    ng = nch // G

    gidx = np.zeros((NCORES, ng, 128, 128), dtype=np.int32)
    dstl = np.full((NCORES, ng, 128, 128), 999.0, dtype=np.float32)
    srow = np.full((NCORES, ng, 128, G), TRASH, dtype=np.int32)
    dinvc = np.zeros((NCORES, ng, 128, G), dtype=np.float32)
    rdivc = np.zeros((NCORES, ng, 1, G * 128), dtype=np.float32)

    for c in range(NCORES):
        chunks, s_sorted, rp, cnt = cores[c]
        v0 = c * SHARD
        for ci, (lo, hi) in enumerate(chunks):
            g, cg = ci // G, ci % G
            e = s_sorted[rp[lo]:rp[hi]]
            ne, span = len(e), hi - lo
            es = np.zeros(CHUNK, dtype=np.int64)
            es[:ne] = _slot(e)
            dl = np.full(CHUNK, 999.0, dtype=np.float32)
            dl[:ne] = np.repeat(np.arange(span), cnt[lo:hi]).astype(np.float32)
            gidx[c, g, :, cg * 8:(cg + 1) * 8] = es.reshape(8, 128).T
            dstl[c, g, :, cg * 8:(cg + 1) * 8] = dl.reshape(8, 128).T
            srow[c, g, :span, cg] = np.arange(lo, hi)
            dinvc[c, g, :span, cg] = dinv[v0 + lo:v0 + hi]
            rdivc[c, g, 0, cg * 128:cg * 128 + span] = rdinv[v0 + lo:v0 + hi]

    return dinv, gidx, dstl, srow, dinvc, rdivc, ng


def _build_program(ng, prelu_a):
    nc = bacc.Bacc("TRN2", target_bir_lowering=False, debug=False,
                   num_devices=NCORES)

    xT = nc.dram_tensor("xT", [128, TROWS], BF16, kind="ExternalInput")
    gidx = nc.dram_tensor("gidx", [ng, 128, 128], I32, kind="ExternalInput")
    dstl = nc.dram_tensor("dstl", [ng, 128, 128], BF16, kind="ExternalInput")
    srow = nc.dram_tensor("srow", [ng, 128, G], I32, kind="ExternalInput")
    dinvc = nc.dram_tensor("dinvc", [ng, 128, G], F32, kind="ExternalInput")
    rdivc = nc.dram_tensor("rdivc", [ng, 1, G * 128], BF16, kind="ExternalInput")
    wts = nc.dram_tensor("wts", [128, 4 * 128], BF16, kind="ExternalInput")
    brows = nc.dram_tensor("brows", [1, 4 * 128], BF16, kind="ExternalInput")
    ident = nc.dram_tensor("ident", [128, 128], BF16, kind="ExternalInput")
    onesr = nc.dram_tensor("onesr", [1, 128], BF16, kind="ExternalInput")
    iot = nc.dram_tensor("iot", [128, 1024], BF16, kind="ExternalInput")

    z_out = nc.dram_tensor("z_out", [LBUF, 128], F32, kind="ExternalOutput")
    p_out = nc.dram_tensor("p_out", [LBUF, 128], F32, kind="ExternalOutput")

    t1 = nc.dram_tensor("t1", [TROWS, 128], BF16)
    t2s = nc.dram_tensor("t2s", [LBUF, 128], BF16)
    t2f = nc.dram_tensor("t2f", [TROWS, 128], BF16)

    with tile.TileContext(nc) as tc:
        with tc.tile_pool(name="const", bufs=1) as cp:
            w_t = cp.tile([128, 4 * 128], BF16)
            nc.sync.dma_start(out=w_t[:], in_=wts[:, :])
            W1, W2 = w_t[:, 0:128], w_t[:, 128:256]
            Wp1, Wp2 = w_t[:, 256:384], w_t[:, 384:512]
            br_t = cp.tile([1, 4 * 128], BF16)
            nc.sync.dma_start(out=br_t[:], in_=brows[:, :])
            id_t = cp.tile([128, 128], BF16)
            nc.sync.dma_start(out=id_t[:], in_=ident[:, :])
            on_t = cp.tile([1, 128], BF16)
            nc.sync.dma_start(out=on_t[:], in_=onesr[:, :])
            io_t = cp.tile([128, 1024], BF16)
            nc.sync.dma_start(out=io_t[:], in_=iot[:, :])

            # ---- phase 1: t1 = (dinv*x) @ W1, full table, built locally ----
            with tc.tile_pool(name="p1", bufs=3) as p1, \
                 tc.tile_pool(name="ps1", bufs=4, space="PSUM") as ps1:
                for bb in range(TROWS // 1024):
                    lx = p1.tile([128, 1024], BF16, tag="lx")
                    nc.sync.dma_start(out=lx[:], in_=xT[:, bb * 1024:(bb + 1) * 1024])
                    stg = p1.tile([128, 1024], BF16, tag="stg")
                    for t in range(8):
                        pt = ps1.tile([128, 128], F32, tag="mm")
                        nc.tensor.matmul(out=pt[:], lhsT=lx[:, t * 128:(t + 1) * 128],
                                         rhs=W1, start=True, stop=True)
                        nc.vector.tensor_copy(out=stg[:, t * 128:(t + 1) * 128], in_=pt[:])
                    for t in range(8):
                        nc.sync.dma_start(
                            out=t1[(bb * 8 + t) * 128:(bb * 8 + t + 1) * 128, :],
                            in_=stg[:, t * 128:(t + 1) * 128])

            def conv(table, is_conv1):
                bias_row = br_t[:, 0:128] if is_conv1 else br_t[:, 128:256]
                with tc.tile_pool(name="cv", bufs=3) as cv, \
                     tc.tile_pool(name="cvg", bufs=2) as cvg, \
                     tc.tile_pool(name="seg", bufs=3, space="PSUM") as segp, \
                     tc.tile_pool(name="aux", bufs=2, space="PSUM") as auxp:
                    for g in range(ng):
                        gi = cvg.tile([128, 128], I32, tag="gi")
                        nc.sync.dma_start(out=gi[:], in_=gidx[g, :, :])
                        dl = cvg.tile([128, 128], BF16, tag="dl")
                        nc.sync.dma_start(out=dl[:], in_=dstl[g, :, :])
                        sr = cvg.tile([128, G], I32, tag="sr")
                        nc.sync.dma_start(out=sr[:], in_=srow[g, :, :])
                        dv = cvg.tile([128, G], F32, tag="dv")
                        nc.sync.dma_start(out=dv[:], in_=dinvc[g, :, :])
                        rd = cvg.tile([1, G * 128], BF16, tag="rd")
                        nc.sync.dma_start(out=rd[:], in_=rdivc[g, :, :])
                        for cg in range(G):
                            msgs = []
                            for t in range(8):
                                git = cv.tile([128, 1], I32, tag=f"git{t}")
                                nc.sync.dma_start(
                                    out=git[:],
                                    in_=gidx[g, :, cg * 8 + t:cg * 8 + t + 1])
                                mt = cv.tile([128, 128], BF16, tag=f"mt{t}")
                                nc.gpsimd.indirect_dma_start(
                                    out=mt[:], out_offset=None, in_=table[:, :],
                                    in_offset=bass.IndirectOffsetOnAxis(
                                        ap=git[:], axis=0))
                                msgs.append(mt)
                            oneh = cv.tile([128, 1024], BF16, tag="oneh")
                            nc.vector.tensor_tensor(
                                out=oneh[:].rearrange("p (j f) -> p j f", f=128),
                                in0=dl[:, cg * 8:(cg + 1) * 8, None].to_broadcast(
                                    [128, 8, 128]),
                                in1=io_t[:].rearrange("p (j f) -> p j f", f=128),
                                op=mybir.AluOpType.is_equal)
                            pt = segp.tile([128, 128], F32, tag="seg")
                            for t in range(8):
                                nc.tensor.matmul(out=pt[:],
                                                 lhsT=oneh[:, t * 128:(t + 1) * 128],
                                                 rhs=msgs[t][:],
                                                 start=(t == 0), stop=False)
                            nc.tensor.matmul(out=pt[:],
                                             lhsT=rd[:, cg * 128:(cg + 1) * 128],
                                             rhs=bias_row, start=False, stop=True)
                            if is_conv1:
                                zr = cv.tile([128, 128], F32, tag="zr")
                                nc.scalar.activation(zr[:], pt[:],
                                                     mybir.ActivationFunctionType.Relu,
                                                     scale=dv[:, cg:cg + 1])
                                z1p = cv.tile([128, 128], BF16, tag="z1p")
                                nc.vector.tensor_scalar_mul(out=z1p[:], in0=zr[:],
                                                            scalar1=dv[:, cg:cg + 1])
                                ptr = auxp.tile([128, 128], BF16, tag="tr")
                                nc.tensor.transpose(out=ptr[:], in_=z1p[:],
                                                    identity=id_t[:])
                                z1pT = cv.tile([128, 128], BF16, tag="z1pT")
                                nc.vector.tensor_copy(out=z1pT[:], in_=ptr[:])
                                pm2 = auxp.tile([128, 128], F32, tag="mm2")
                                nc.tensor.matmul(out=pm2[:], lhsT=z1pT[:], rhs=W2,
                                                 start=True, stop=True)
                                t2r = cv.tile([128, 128], BF16, tag="t2r")
                                nc.vector.tensor_copy(out=t2r[:], in_=pm2[:])
                                srt = cv.tile([128, 1], I32, tag="srt")
                                nc.sync.dma_start(out=srt[:],
                                                  in_=srow[g, :, cg:cg + 1])
                                nc.gpsimd.indirect_dma_start(
                                    out=t2s[:, :],
                                    out_offset=bass.IndirectOffsetOnAxis(
                                        ap=srt[:], axis=0),
                                    in_=t2r[:], in_offset=None)
                            else:
                                zf = cv.tile([128, 128], F32, tag="zf")
                                nc.scalar.activation(zf[:], pt[:],
                                                     mybir.ActivationFunctionType.Relu,
                                                     scale=dv[:, cg:cg + 1])
                                srt = cv.tile([128, 1], I32, tag="srt")
                                nc.sync.dma_start(out=srt[:],
                                                  in_=srow[g, :, cg:cg + 1])
                                nc.gpsimd.indirect_dma_start(
                                    out=z_out[:, :],
                                    out_offset=bass.IndirectOffsetOnAxis(
                                        ap=srt[:], axis=0),
                                    in_=zf[:], in_offset=None)

            conv(t1, True)

            nc.gpsimd.collective_compute(
                "AllGather", mybir.AluOpType.bypass,
                replica_groups=[list(range(NCORES))],
                ins=[t2s[0:SLOTS, :].opt()], outs=[t2f[:, :].opt()])

            conv(t2f, False)

            # ---- projection head: p = prelu(z@Wp1+bp1, a) @ Wp2 + bp2 ----
            with tc.tile_pool(name="pj", bufs=3) as pj, \
                 tc.tile_pool(name="pjp", bufs=2, space="PSUM") as pjp:
                for b in range(NBLK):
                    zl = pj.tile([128, 128], F32, tag="zl")
                    nc.sync.dma_start(out=zl[:], in_=z_out[b * 128:(b + 1) * 128, :])
                    zb = pj.tile([128, 128], BF16, tag="zb")
                    nc.vector.tensor_copy(out=zb[:], in_=zl[:])
                    ptr = pjp.tile([128, 128], BF16, tag="tr")
                    nc.tensor.transpose(out=ptr[:], in_=zb[:], identity=id_t[:])
                    zT = pj.tile([128, 128], BF16, tag="zT")
                    nc.vector.tensor_copy(out=zT[:], in_=ptr[:])
                    ph = pjp.tile([128, 128], F32, tag="mm")
                    nc.tensor.matmul(out=ph[:], lhsT=zT[:], rhs=Wp1,
                                     start=True, stop=False)
                    nc.tensor.matmul(out=ph[:], lhsT=on_t[:], rhs=br_t[:, 256:384],
                                     start=False, stop=True)
                    pos = pj.tile([128, 128], F32, tag="pos")
                    nc.scalar.activation(pos[:], ph[:],
                                         mybir.ActivationFunctionType.Relu)
                    neg = pj.tile([128, 128], F32, tag="neg")
                    nc.vector.tensor_scalar(out=neg[:], in0=ph[:], scalar1=0.0,
                                            scalar2=float(prelu_a),
                                            op0=mybir.AluOpType.min,
                                            op1=mybir.AluOpType.mult)
                    h3 = pj.tile([128, 128], BF16, tag="h3")
                    nc.vector.tensor_add(out=h3[:], in0=pos[:], in1=neg[:])
                    ptr2 = pjp.tile([128, 128], BF16, tag="tr2")
                    nc.tensor.transpose(out=ptr2[:], in_=h3[:], identity=id_t[:])
                    h3T = pj.tile([128, 128], BF16, tag="h3T")
                    nc.vector.tensor_copy(out=h3T[:], in_=ptr2[:])
                    pp = pjp.tile([128, 128], F32, tag="mmp")
                    nc.tensor.matmul(out=pp[:], lhsT=h3T[:], rhs=Wp2,
                                     start=True, stop=False)
                    nc.tensor.matmul(out=pp[:], lhsT=on_t[:], rhs=br_t[:, 384:512],
                                     start=False, stop=True)
                    pf = pj.tile([128, 128], F32, tag="pf")
                    nc.vector.tensor_copy(out=pf[:], in_=pp[:])
                    nc.sync.dma_start(out=p_out[b * 128:(b + 1) * 128, :], in_=pf[:])

    nc.compile()
    return nc


def kernel(x, edge_index, W1, b1, W2, b2, Wp1, bp1, prelu_a, Wp2, bp2,
           _timing=None):
    x = np.asarray(x, dtype=np.float32)
    dinv, gidx, dstl, srow, dinvc, rdivc, ng = _preprocess(edge_index)

    xs = x * dinv[:, None]
    x_slot = np.zeros((TROWS, 128), dtype=np.float32)
    x_slot[_slot(np.arange(N))] = xs
    xT_np = np.ascontiguousarray(x_slot.T).astype(BF)

    wts_np = np.concatenate(
        [np.asarray(w, np.float32) for w in (W1, W2, Wp1, Wp2)], axis=1).astype(BF)
    brows_np = np.concatenate(
        [np.asarray(b, np.float32).reshape(1, 128) for b in (b1, b2, bp1, bp2)],
        axis=1).astype(BF)
    ident_np = np.eye(128, dtype=np.float32).astype(BF)
    ones_np = np.ones((1, 128), dtype=np.float32).astype(BF)
    iot_np = np.tile(np.arange(128, dtype=np.float32), 8)[None, :].repeat(
        128, 0).astype(BF)

    nc = _build_program(ng, float(np.asarray(prelu_a)))

    in_maps = []
    for c in range(NCORES):
        in_maps.append({
            "xT": xT_np,
            "gidx": gidx[c], "dstl": dstl[c].astype(BF),
            "srow": srow[c], "dinvc": dinvc[c], "rdivc": rdivc[c].astype(BF),
            "wts": wts_np, "brows": brows_np, "ident": ident_np,
            "onesr": ones_np, "iot": iot_np,
        })

    kwargs = dict(_timing.get("kwargs", {})) if _timing else {}
    res = run_bass_kernel_spmd(nc, in_maps, core_ids=list(range(NCORES)), **kwargs)
    if _timing is not None:
        _timing["exec_time_ns"] = res.exec_time_ns

    z = np.concatenate([res.results[c]["z_out"][:SHARD] for c in range(NCORES)],
                       axis=0)
    p = np.concatenate([res.results[c]["p_out"][:SHARD] for c in range(NCORES)],
                       axis=0)
    return (z, p)



# revision 8
# speedup vs baseline: 1.7186x; 1.7186x over previous
"""GCN (2x GCNConv + ReLU, then Linear+PReLU+Linear) on 8 Trainium2 NeuronCores.

Sharding: destination-node range partitioning (12500 nodes/core, padded to
12544 = 98 windows of 128). Aggregation commutes with the weight matmul, so
the conv1 gather table is just dinv*x built on the host (no on-device table
build), and W1/W2 apply post-aggregation to the 12.5k-row shard. Edges are
bucketed by (dst window, source quarter-range) with 4 tiles of 128 slots per
bucket; each group of 7 windows is fetched with 4 dma_gather instructions
(int16 indices relative to the quarter base), then segment-summed via one-hot
matmuls into PSUM per window. Epilogues use direct window-aligned stores; the
projection head is fused into conv2. The layer-2 table shard t2 = dinv*z1 is
AllGathered between convs. Tables are bf16; accumulation is f32 in PSUM.
"""

import sys

sys.path.insert(0, "/opt/trn_rl_repo")

import numpy as np
import ml_dtypes

import concourse.bass as bass
import concourse.bacc as bacc
import concourse.tile as tile
from concourse import mybir
from concourse.bass_utils import run_bass_kernel_spmd

N = 100000
E = 1000000
D = 128
NCORES = 8
SHARD = N // NCORES          # 12500
SLOTS = 12544                # 98 * 128 per-core slot space (padded)
TROWS = NCORES * SLOTS       # 100352 table rows
NW = SLOTS // 128            # 98 windows of 128 dst nodes
GW = 7                       # windows per gather group
NG = NW // GW                # 14 groups
NR = 4                       # source quarter-ranges (int16-addressable)
RSIZE = TROWS // NR          # 25088 rows per range
T = 4                        # tiles of 128 edge slots per (window, range)
BUCKET = T * 128             # 512 edge slots per bucket
GIDX = GW * T * 128          # 3584 idxs per (group, range) gather
GCOL = GIDX // 16            # 224 wrapped idx columns per (group, range)

BF16 = mybir.dt.bfloat16
F32 = mybir.dt.float32
I16 = mybir.dt.int16
BF = ml_dtypes.bfloat16


def _slot(v):
    return (v // SHARD) * SLOTS + (v % SHARD)


def _preprocess(edge_index):
    """Host-side graph prep. Returns dinv + per-core constant arrays."""
    src = np.asarray(edge_index[0], dtype=np.int64)
    dst = np.asarray(edge_index[1], dtype=np.int64)
    deg = np.bincount(dst, minlength=N).astype(np.float64) + 1.0  # + self loop
    dinv = (1.0 / np.sqrt(deg)).astype(np.float32)
    rdinv = np.sqrt(deg).astype(np.float32)

    gidx16 = np.zeros((NCORES, 16, NR * NG * GCOL), dtype=np.int16)
    dstl = np.full((NCORES, 128, NW * NR * T), 999.0, dtype=np.float32)
    dinvc = np.zeros((NCORES, 128, NW), dtype=np.float32)
    rdivc = np.zeros((NCORES, 1, SLOTS), dtype=np.float32)

    for c in range(NCORES):
        v0, v1 = c * SHARD, (c + 1) * SHARD
        m = (dst >= v0) & (dst < v1)
        loops = np.arange(v0, v1, dtype=np.int64)
        s_all = np.concatenate([src[m], loops])
        d_all = np.concatenate([dst[m], loops])
        ldst = d_all - v0
        w_all = ldst >> 7
        lab = ldst & 127
        slot_s = _slot(s_all)
        r_all = slot_s // RSIZE
        ridx = slot_s % RSIZE

        key = w_all * NR + r_all
        order = np.argsort(key, kind="stable")
        key_s = key[order]
        lab_s = lab[order]
        ridx_s = ridx[order]
        cnt = np.bincount(key_s, minlength=NW * NR)
        assert cnt.max() <= BUCKET, f"bucket overflow: {cnt.max()} > {BUCKET}"
        starts = np.zeros(NW * NR, dtype=np.int64)
        starts[1:] = np.cumsum(cnt)[:-1]
        q = np.arange(len(key_s)) - starts[key_s]          # position in bucket
        w_s = key_s // NR
        r_s = key_s % NR
        s_tile = q >> 7
        p = q & 127
        g = w_s // GW
        lw = w_s % GW
        i_stream = (lw * T + s_tile) * 128 + p             # pos in (g,r) gather
        col = r_s * (NG * GCOL) + g * GCOL + (i_stream >> 4)
        row16 = i_stream & 15
        gidx16[c, row16, col] = ridx_s
        dstl[c, p, w_s * (NR * T) + r_s * T + s_tile] = lab_s
        nodes = np.arange(SHARD)
        dinvc[c][nodes % 128, nodes // 128] = dinv[v0 + nodes]
        rdivc[c, 0, :SHARD] = rdinv[v0:v1]

    gidx_rep = np.tile(gidx16, (1, 8, 1))                  # [NCORES,128,cols]
    return dinv, gidx_rep, dstl, dinvc, rdivc


def _build_program(prelu_a):
    nc = bacc.Bacc("TRN2", target_bir_lowering=False, debug=False,
                   num_devices=NCORES, num_swdge_queues=NR)

    xs = nc.dram_tensor("xs", [TROWS, 128], BF16, kind="ExternalInput")
    gidx = nc.dram_tensor("gidx", [128, NR * NG * GCOL], I16,
                          kind="ExternalInput")
    dstl = nc.dram_tensor("dstl", [128, NW * NR * T], BF16,
                          kind="ExternalInput")
    dinvc = nc.dram_tensor("dinvc", [128, NW], F32, kind="ExternalInput")
    rdivc = nc.dram_tensor("rdivc", [1, SLOTS], BF16, kind="ExternalInput")
    wts = nc.dram_tensor("wts", [128, 4 * 128], BF16, kind="ExternalInput")
    brows = nc.dram_tensor("brows", [1, 4 * 128], BF16, kind="ExternalInput")
    ident = nc.dram_tensor("ident", [128, 128], BF16, kind="ExternalInput")
    onesr = nc.dram_tensor("onesr", [1, 128], BF16, kind="ExternalInput")
    iot = nc.dram_tensor("iot", [128, NR * T * 128], BF16,
                         kind="ExternalInput")

    z_out = nc.dram_tensor("z_out", [SLOTS, 128], F32, kind="ExternalOutput")
    p_out = nc.dram_tensor("p_out", [SLOTS, 128], F32, kind="ExternalOutput")

    t2s = nc.dram_tensor("t2s", [SLOTS, 128], BF16)
    t2f = nc.dram_tensor("t2f", [TROWS, 128], BF16, addr_space="Shared")

    with tile.TileContext(nc) as tc:
        with tc.tile_pool(name="const", bufs=1) as cp:
            w_t = cp.tile([128, 4 * 128], BF16)
            nc.sync.dma_start(out=w_t[:], in_=wts[:, :])
            W1, W2 = w_t[:, 0:128], w_t[:, 128:256]
            Wp1, Wp2 = w_t[:, 256:384], w_t[:, 384:512]
            br_t = cp.tile([1, 4 * 128], BF16)
            nc.sync.dma_start(out=br_t[:], in_=brows[:, :])
            id_t = cp.tile([128, 128], BF16)
            nc.sync.dma_start(out=id_t[:], in_=ident[:, :])
            on_t = cp.tile([1, 128], BF16)
            nc.sync.dma_start(out=on_t[:], in_=onesr[:, :])
            io_t = cp.tile([128, NR * T * 128], BF16)
            nc.sync.dma_start(out=io_t[:], in_=iot[:, :])
            gi_t = cp.tile([128, NR * NG * GCOL], I16)
            nc.sync.dma_start(out=gi_t[:], in_=gidx[:, :])
            dl_t = cp.tile([128, NW * NR * T], BF16)
            nc.sync.dma_start(out=dl_t[:], in_=dstl[:, :])
            dv_t = cp.tile([128, NW], F32)
            nc.sync.dma_start(out=dv_t[:], in_=dinvc[:, :])
            rd_t = cp.tile([1, SLOTS], BF16)
            nc.sync.dma_start(out=rd_t[:], in_=rdivc[:, :])

            def conv(table, is_conv1):
                bias_row = br_t[:, 0:128] if is_conv1 else br_t[:, 128:256]
                Wmain = W1 if is_conv1 else W2
                with tc.tile_pool(name="gat", bufs=2) as gat, \
                     tc.tile_pool(name="wk", bufs=3) as wk, \
                     tc.tile_pool(name="agp", bufs=2, space="PSUM") as agp, \
                     tc.tile_pool(name="aux", bufs=2, space="PSUM") as auxp:
                    for g in range(NG):
                        gout = gat.tile([128, NR * GW * T * 128], BF16,
                                        tag="gout")
                        # Q7 scratch caps one dma_gather at ~1024 idxs, so
                        # each (group, range) stream issues as 4 sub-calls of
                        # 896; queue_num=r spreads emission over the 4 Q7
                        # pairs.
                        SUB = GIDX // 4              # 896 idxs per call
                        SUBC = SUB // 16             # 56 wrapped columns
                        for r in range(NR):
                            for k in range(4):
                                nc.gpsimd.dma_gather(
                                    out_ap=gout[:, r * GIDX + k * SUB:
                                                r * GIDX + (k + 1) * SUB]
                                    .rearrange("p (j e) -> p j e", e=128),
                                    in_ap=table[r * RSIZE:(r + 1) * RSIZE, :],
                                    idxs_ap=gi_t[:, (r * NG + g) * GCOL
                                                 + k * SUBC:
                                                 (r * NG + g) * GCOL
                                                 + (k + 1) * SUBC],
                                    num_idxs=SUB,
                                    num_idxs_reg=SUB,
                                    elem_size=128,
                                    queue_num=r,
                                )
                        for lw in range(GW):
                            w = g * GW + lw
                            oneh = wk.tile([128, NR * T * 128], BF16,
                                           tag="oneh")
                            nc.vector.tensor_tensor(
                                out=oneh[:].rearrange("p (j f) -> p j f",
                                                      f=128),
                                in0=dl_t[:, w * NR * T:(w + 1) * NR * T, None]
                                .to_broadcast([128, NR * T, 128]),
                                in1=io_t[:].rearrange("p (j f) -> p j f",
                                                      f=128),
                                op=mybir.AluOpType.is_equal)
                            agg = agp.tile([128, 128], F32, tag="agg")
                            for t in range(NR * T):
                                r, s = t // T, t % T
                                msg = gout[:, (r * GW * T + lw * T + s) * 128:
                                           (r * GW * T + lw * T + s + 1) * 128]
                                nc.tensor.matmul(
                                    out=agg[:], lhsT=msg,
                                    rhs=oneh[:, t * 128:(t + 1) * 128],
                                    start=(t == 0), stop=(t == NR * T - 1))
                            aggb = wk.tile([128, 128], BF16, tag="aggb")
                            nc.vector.tensor_copy(out=aggb[:], in_=agg[:])
                            zp = auxp.tile([128, 128], F32, tag="mm")
                            nc.tensor.matmul(out=zp[:], lhsT=aggb[:],
                                             rhs=Wmain, start=True, stop=False)
                            nc.tensor.matmul(
                                out=zp[:],
                                lhsT=rd_t[:, w * 128:(w + 1) * 128],
                                rhs=bias_row, start=False, stop=True)
                            dv = dv_t[:, w:w + 1]
                            if is_conv1:
                                z1 = wk.tile([128, 128], F32, tag="z1")
                                nc.scalar.activation(
                                    z1[:], zp[:],
                                    mybir.ActivationFunctionType.Relu,
                                    scale=dv)
                                t2r = wk.tile([128, 128], BF16, tag="t2r")
                                nc.vector.tensor_scalar_mul(
                                    out=t2r[:], in0=z1[:], scalar1=dv)
                                nc.sync.dma_start(
                                    out=t2s[w * 128:(w + 1) * 128, :],
                                    in_=t2r[:])
                            else:
                                zf = wk.tile([128, 128], F32, tag="zf")
                                nc.scalar.activation(
                                    zf[:], zp[:],
                                    mybir.ActivationFunctionType.Relu,
                                    scale=dv)
                                nc.sync.dma_start(
                                    out=z_out[w * 128:(w + 1) * 128, :],
                                    in_=zf[:])
                                zb = wk.tile([128, 128], BF16, tag="zb")
                                nc.vector.tensor_copy(out=zb[:], in_=zf[:])
                                ztp = auxp.tile([128, 128], BF16, tag="tr")
                                nc.tensor.transpose(out=ztp[:], in_=zb[:],
                                                    identity=id_t[:])
                                ztb = wk.tile([128, 128], BF16, tag="ztb")
                                nc.vector.tensor_copy(out=ztb[:], in_=ztp[:])
                                hp = auxp.tile([128, 128], F32, tag="mm")
                                nc.tensor.matmul(out=hp[:], lhsT=ztb[:],
                                                 rhs=Wp1, start=True,
                                                 stop=False)
                                nc.tensor.matmul(out=hp[:], lhsT=on_t[:],
                                                 rhs=br_t[:, 256:384],
                                                 start=False, stop=True)
                                pos = wk.tile([128, 128], F32, tag="pos")
                                nc.scalar.activation(
                                    pos[:], hp[:],
                                    mybir.ActivationFunctionType.Relu)
                                neg = wk.tile([128, 128], F32, tag="neg")
                                nc.vector.tensor_scalar(
                                    out=neg[:], in0=hp[:], scalar1=0.0,
                                    scalar2=float(prelu_a),
                                    op0=mybir.AluOpType.min,
                                    op1=mybir.AluOpType.mult)
                                h3 = wk.tile([128, 128], BF16, tag="h3")
                                nc.vector.tensor_add(out=h3[:], in0=pos[:],
                                                     in1=neg[:])
                                htp = auxp.tile([128, 128], BF16, tag="tr")
                                nc.tensor.transpose(out=htp[:], in_=h3[:],
                                                    identity=id_t[:])
                                htb = wk.tile([128, 128], BF16, tag="htb")
                                nc.vector.tensor_copy(out=htb[:], in_=htp[:])
                                pp = auxp.tile([128, 128], F32, tag="mm")
                                nc.tensor.matmul(out=pp[:], lhsT=htb[:],
                                                 rhs=Wp2, start=True,
                                                 stop=False)
                                nc.tensor.matmul(out=pp[:], lhsT=on_t[:],
                                                 rhs=br_t[:, 384:512],
                                                 start=False, stop=True)
                                pf = wk.tile([128, 128], F32, tag="pf")
                                nc.vector.tensor_copy(out=pf[:], in_=pp[:])
                                nc.sync.dma_start(
                                    out=p_out[w * 128:(w + 1) * 128, :],
                                    in_=pf[:])

            conv(xs, True)

            nc.gpsimd.collective_compute(
                "AllGather", mybir.AluOpType.bypass,
                replica_groups=[list(range(NCORES))],
                ins=[t2s[:, :].opt()], outs=[t2f[:, :].opt()])

            conv(t2f, False)

    nc.compile()
    return nc


def kernel(x, edge_index, W1, b1, W2, b2, Wp1, bp1, prelu_a, Wp2, bp2,
           _timing=None):
    x = np.asarray(x, dtype=np.float32)
    dinv, gidx_rep, dstl, dinvc, rdivc = _preprocess(edge_index)

    xs_full = x * dinv[:, None]
    x_slot = np.zeros((TROWS, 128), dtype=np.float32)
    x_slot[_slot(np.arange(N))] = xs_full
    xs_np = x_slot.astype(BF)

    wts_np = np.concatenate(
        [np.asarray(w, np.float32) for w in (W1, W2, Wp1, Wp2)],
        axis=1).astype(BF)
    brows_np = np.concatenate(
        [np.asarray(b, np.float32).reshape(1, 128) for b in (b1, b2, bp1, bp2)],
        axis=1).astype(BF)
    ident_np = np.eye(128, dtype=np.float32).astype(BF)
    ones_np = np.ones((1, 128), dtype=np.float32).astype(BF)
    iot_np = np.tile(np.arange(128, dtype=np.float32), NR * T)[None, :].repeat(
        128, 0).astype(BF)

    nc = _build_program(float(np.asarray(prelu_a)))

    in_maps = []
    for c in range(NCORES):
        in_maps.append({
            "xs": xs_np,
            "gidx": gidx_rep[c], "dstl": dstl[c].astype(BF),
            "dinvc": dinvc[c], "rdivc": rdivc[c].astype(BF),
            "wts": wts_np, "brows": brows_np, "ident": ident_np,
            "onesr": ones_np, "iot": iot_np,
        })

    kwargs = dict(_timing.get("kwargs", {})) if _timing else {}
    res = run_bass_kernel_spmd(nc, in_maps, core_ids=list(range(NCORES)),
                               **kwargs)
    if _timing is not None:
        _timing["exec_time_ns"] = res.exec_time_ns

    z = np.concatenate([res.results[c]["z_out"][:SHARD]
                        for c in range(NCORES)], axis=0)
    p = np.concatenate([res.results[c]["p_out"][:SHARD]
                        for c in range(NCORES)], axis=0)
    return (z, p)
